# revision 12
# baseline (speedup 1.0000x reference)
"""Custom LSTM-cell kernel for Trainium2, data-parallel over batch on 8 NeuronCores.

Math (per token, elementwise over dff except the two GEMMs):
    gates = Hi @ Wh + Zi @ Wz + bias         # [tok, 4*dff], gate order I|F|O|Z
    A   = F~ + Mi
    M_t = max(A, I~)
    Dd  = A - I~
    F_t = exp(min(Dd, 0))
    I_t = exp(min(-Dd, 0))
    O_t = sigmoid(O~) = 0.5*(1 + tanh(O~/2))
    Z_t = tanh(Z~)
    N_t = F_t*Ni + I_t
    C_t = (Ci*F_t + Z_t*I_t)*m + (1-m)*Ci = Ci*(F_t*m + 1-m) + (Z_t*I_t)*m
    H_t = O_t*(C_t/N_t)*m + (1-m)*Hi = (tanh(O~/2)+1)*(C_t/N_t)*(0.5m) + (1-m)*Hi

Device layout: tokens on partitions, gate columns on the free dim. Activations
are pre-transposed on host to fp16 [dff, tok] as the stationary matmul operand;
weights are the moving operand (fp16 in, fp32 PSUM accumulate). All four gate
biases are seeded into PSUM by a K=1 ones-row matmul. All elementwise traffic
(Mi/Ci/Ni/(1-m)Hi in, all four outputs out) is fp16 to halve HBM bytes; the
host upcasts outputs to fp32. The exp arguments are formed from a single
fp32 PSUM difference (Dd) so no catastrophic cancellation passes through
fp16. Engine balance: PSUM-reading ops + STT fusions on DVE, the four
transcendentals + the m-blend (scale/bias) on ScalarE, four streaming
multiplies/adds on GPSIMD, keeping every engine under the PE matmul stream.
"""

import numpy as np

import concourse.bass as bass
import concourse.tile as tile
import concourse.bass_utils as bass_utils
from concourse import bacc, mybir
from concourse.bass import ts, ds

B, P, D, DFF = 256, 64, 512, 1024
NCORES = 8
BL = B // NCORES          # batches per core
TOK = BL * P              # tokens per core (2048)
NT = TOK // 128           # token tiles per core (16)
KH = DFF // 128           # Hi k-tiles (8)
KZ = D // 128             # Zi k-tiles (4)
KT = KH + KZ              # total k-tiles (12)
CH = 2                    # dff column chunks of 512 per gate
CW = 512                  # chunk width

F32 = mybir.dt.float32
F16 = mybir.dt.float16
AF = mybir.ActivationFunctionType
OP = mybir.AluOpType

_CACHE = {}
THREAD_INPUT = "mpk"   # small input used by test.py to chain timed executions


def _build(repeat: int = 1, climit: int = CH, tlimit: int = NT):
    """Build + compile the per-core Bass module. Cached per config."""
    key = (repeat, climit, tlimit)
    if key in _CACHE:
        return _CACHE[key]

    nc = bacc.Bacc("TRN2", target_bir_lowering=False, debug=False,
                   num_devices=NCORES)

    hiT = nc.dram_tensor("hiT", [KH, 128, TOK], F16, kind="ExternalInput").ap()
    ziT = nc.dram_tensor("ziT", [KZ, 128, TOK], F16, kind="ExternalInput").ap()
    w = nc.dram_tensor("w", [CH, KT, 128, 4, CW], F16, kind="ExternalInput").ap()
    bias = nc.dram_tensor("bias", [1, CH, 4, CW], F16, kind="ExternalInput").ap()
    biasb = nc.dram_tensor("biasb", [1, CH, CW], F16, kind="ExternalInput").ap()
    elin = nc.dram_tensor("elin", [NT, 128, 4, CH, CW], F16,
                          kind="ExternalInput").ap()
    mpk = nc.dram_tensor("mpk", [NT, 128, 3], F32, kind="ExternalInput").ap()

    elout = nc.dram_tensor("elout", [NT, 128, 4, CH, CW], F16,
                           kind="ExternalOutput").ap()

    with tile.TileContext(nc) as tc:
        with (
            tc.tile_pool(name="singles", bufs=1) as singles,
            tc.tile_pool(name="wpool", bufs=KT + 4) as wpool,
            tc.tile_pool(name="inpool", bufs=3) as inpool,
            tc.tile_pool(name="tmpA", bufs=2) as tmpA,
            tc.tile_pool(name="tmpB", bufs=2) as tmpB,
            tc.tile_pool(name="outp", bufs=2) as outp,
            tc.tile_pool(name="ps", bufs=8, space="PSUM") as pspool,
        ):
            hiT_sb = []
            for k in range(KH):
                hk = singles.tile([128, TOK], F16, name=f"hiT{k}")
                nc.sync.dma_start(out=hk, in_=hiT[k])
                hiT_sb.append(hk)
            ziT_sb = []
            for k in range(KZ):
                zk = singles.tile([128, TOK], F16, name=f"ziT{k}")
                nc.sync.dma_start(out=zk, in_=ziT[k])
                ziT_sb.append(zk)
            mpk_sb = singles.tile([128, NT, 3], F32)
            nc.sync.dma_start(out=mpk_sb, in_=mpk.rearrange("t p c -> p t c"))
            bias_sb = singles.tile([1, CH, 4, CW], F16)
            nc.sync.dma_start(out=bias_sb, in_=bias)
            ones_sb = singles.tile([1, 128], F16)
            nc.vector.memset(ones_sb, 1.0)
            bb_sb = singles.tile([128, 1, CH, CW], F16)
            for cj in range(CH):
                bsl = biasb[0, cj]
                bcast = bass.AP(tensor=bsl.tensor, offset=bsl.offset,
                                ap=[[0, 128]] + list(bsl.ap))
                nc.gpsimd.dma_start(out=bb_sb[:, 0, cj], in_=bcast)

            for _ in range(repeat):
                def phase2(st):
                    (c, Dd, mx, th, Zt, ci_t, ni_t, ho_t,
                     m_ap, om_ap, hm_ap) = st
                    ot = outp.tile([128, 4, CW], F16, tag="ot")
                    Mt = ot[:, 1]
                    nc.vector.tensor_add(Mt, mx, bb_sb[:, 0, c])
                    s1 = tmpA.tile([128, CW], F16, tag="s1")
                    nc.vector.tensor_scalar_min(s1, Dd, 0.0)
                    s2 = tmpA.tile([128, CW], F16, tag="s2")
                    nc.vector.tensor_scalar(s2, Dd, -1.0, 0.0, OP.mult, OP.min)
                    Ft = tmpB.tile([128, CW], F16, tag="Ft")
                    nc.scalar.activation(Ft, s1, AF.Exp)
                    It = tmpB.tile([128, CW], F16, tag="It")
                    nc.scalar.activation(It, s2, AF.Exp)

                    FN = tmpA.tile([128, CW], F16, tag="FN")
                    nc.gpsimd.tensor_mul(FN, Ft, ni_t)
                    Nt = tmpB.tile([128, CW], F32, tag="Nt")
                    nc.vector.tensor_add(Nt, FN, It)
                    nc.scalar.activation(ot[:, 3], Nt, AF.Copy)
                    rec = tmpB.tile([128, CW], F32, tag="rec")
                    nc.vector.reciprocal_approx_fast(rec, Nt)
                    mF = tmpA.tile([128, CW], F16, tag="mF")
                    nc.scalar.activation(mF, Ft, AF.Identity,
                                         scale=m_ap, bias=om_ap)
                    p1 = tmpA.tile([128, CW], F16, tag="p1")
                    nc.gpsimd.tensor_mul(p1, ci_t, mF)
                    t2 = tmpA.tile([128, CW], F16, tag="t2")
                    nc.gpsimd.tensor_mul(t2, Zt, It)
                    Ct = ot[:, 0]
                    nc.vector.scalar_tensor_tensor(Ct, t2, m_ap, p1,
                                                   OP.mult, OP.add)
                    R1 = tmpA.tile([128, CW], F16, tag="R1")
                    nc.gpsimd.tensor_mul(R1, rec, Ct)
                    u = tmpA.tile([128, CW], F16, tag="u")
                    nc.vector.scalar_tensor_tensor(u, th, 1.0, R1,
                                                   OP.add, OP.mult)
                    Ht = ot[:, 2]
                    nc.vector.scalar_tensor_tensor(Ht, u, hm_ap, ho_t,
                                                   OP.mult, OP.add)

                    tt, cc = st_tc[id(st)]
                    nc.sync.dma_start(out=elout[tt, :, :, cc], in_=ot)

                st_tc = {}
                prev = None
                for c in range(climit):
                    wk = []
                    for k in range(KT):
                        wt = wpool.tile([128, 4, CW], F16, tag="wk")
                        nc.sync.dma_start(out=wt, in_=w[c, k])
                        wk.append(wt)
                    for t in range(tlimit):
                        rows = ts(t, 128)
                        cols = ds(c * CW, CW)
                        el_t = inpool.tile([128, 4, CW], F16, tag="el")
                        nc.sync.dma_start(out=el_t, in_=elin[t, :, :, c])
                        mi_t, ci_t, ni_t, ho_t = (el_t[:, 0], el_t[:, 1],
                                                  el_t[:, 2], el_t[:, 3])
                        m_ap = mpk_sb[:, t, 0:1]
                        om_ap = mpk_sb[:, t, 1:2]
                        hm_ap = mpk_sb[:, t, 2:3]

                        ps = [pspool.tile([128, CW], F32, tag="ps", name=f"ps{g}")
                              for g in range(4)]
                        # O/Z bias via K=1 ones-row matmul; I-bias added
                        # post-max; F-bias folded into Mi on host
                        for g in (2, 3):
                            nc.tensor.matmul(ps[g], ones_sb, bias_sb[0:1, c, g],
                                             start=True, stop=False)
                        for k in range(KT):
                            lhsT = (hiT_sb[k][:, rows] if k < KH
                                    else ziT_sb[k - KH][:, rows])
                            for g in range(4):
                                nc.tensor.matmul(ps[g], lhsT, wk[k][:, g],
                                                 start=(k == 0 and g < 2),
                                                 stop=(k == KT - 1))

                        psI, psF, psO, psZ = ps
                        # phase 1: drain PSUM banks immediately
                        A = tmpA.tile([128, CW], F32, tag="A")
                        nc.vector.tensor_add(A, psF, mi_t)
                        Dd = tmpA.tile([128, CW], F16, tag="Dd")
                        nc.vector.tensor_sub(Dd, A, psI)
                        mx = tmpA.tile([128, CW], F16, tag="mx")
                        nc.vector.tensor_max(mx, A, psI)
                        th = tmpB.tile([128, CW], F16, tag="th")
                        nc.scalar.activation(th, psO, AF.Tanh, scale=0.5)
                        Zt = tmpB.tile([128, CW], F16, tag="Zt")
                        nc.scalar.activation(Zt, psZ, AF.Tanh)

                        st = (c, Dd, mx, th, Zt, ci_t, ni_t, ho_t,
                              m_ap, om_ap, hm_ap)
                        st_tc[id(st)] = (t, c)
                        # phase 2 of the previous tile runs behind this
                        # tile's PSUM drains in every engine queue
                        if prev is not None:
                            phase2(prev)
                        prev = st
                if prev is not None:
                    phase2(prev)
                    prev = None

    nc.compile()
    _CACHE[key] = nc
    return nc


def _prep_inputs(inputs):
    """Host-side shard + reformat. Returns per-core input maps."""
    f32, f16 = np.float32, np.float16
    g = {k: np.asarray(v) for k, v in inputs.items()}

    Wh = np.concatenate([g['WI_w'], g['WF_w'], g['WO_w'], g['WZ_w']], axis=1)
    Wz = np.concatenate([g['RI_w'], g['RF_w'], g['RO_w'], g['RZ_w']], axis=1)
    bias = np.concatenate([g['WI_b'] + g['RI_b'], g['WF_b'] + g['RF_b'],
                           g['WO_b'] + g['RO_b'], g['WZ_b'] + g['RZ_b']])
    Wcat = np.vstack([Wh, Wz]).astype(f16)                       # [1536, 4096]
    w_l = np.ascontiguousarray(
        Wcat.reshape(KT, 128, 4, CH, CW).transpose(3, 0, 1, 2, 4))
    bias_l = np.ascontiguousarray(
        bias.astype(f16).reshape(1, 4, CH, CW).transpose(0, 2, 1, 3))
    bI, bF = bias.reshape(4, DFF)[0], bias.reshape(4, DFF)[1]
    biasb_l = np.ascontiguousarray(bI.astype(f16).reshape(1, CH, CW))
    mi_shift = (bF - bI)[None, :]

    in_maps = []
    for c in range(NCORES):
        sl = slice(c * BL, (c + 1) * BL)
        Hi_c = g['Hi'][sl].reshape(TOK, DFF)
        Zi_c = g['Zi'][sl].reshape(TOK, D)
        m_c = g['m'][sl].reshape(TOK, 1).astype(f32)
        hiT = np.ascontiguousarray(Hi_c.T).astype(f16).reshape(KH, 128, TOK)
        ziT = np.ascontiguousarray(Zi_c.T).astype(f16).reshape(KZ, 128, TOK)
        mpk = np.concatenate([m_c, 1.0 - m_c, 0.5 * m_c],
                             axis=1).astype(f32).reshape(NT, 128, 3)
        in_maps.append({
            "hiT": hiT,
            "ziT": ziT,
            "w": w_l,
            "bias": bias_l,
            "biasb": biasb_l,
            "elin": np.ascontiguousarray(np.stack([
                (g['Mi'][sl].reshape(TOK, DFF) + mi_shift).astype(f16),
                g['Ci'][sl].reshape(TOK, DFF).astype(f16),
                g['Ni'][sl].reshape(TOK, DFF).astype(f16),
                ((1.0 - m_c) * Hi_c).astype(f16),
            ], axis=1).reshape(NT, 128, 4, CH, CW)),
            "mpk": mpk,
        })
    return in_maps


def _gather(results):
    # elout[NT, 128, 4(C|M|H|N), CH, CW] fp16 per core -> 4 full fp32 outputs
    def cat(idx):
        full = np.concatenate(
            [results[c]["elout"][:, :, idx]
             .reshape(TOK, DFF).reshape(BL, P, DFF)
             for c in range(NCORES)], axis=0)
        return np.ascontiguousarray(full, dtype=np.float32)
    return cat(0), cat(1), cat(2), cat(3)


def kernel(**inputs):
    nc = _build(repeat=1)
    in_maps = _prep_inputs(inputs)
    res = bass_utils.run_bass_kernel_spmd(nc, in_maps,
                                          core_ids=list(range(NCORES)))
    return _gather(res.results)
